# revision 16
# baseline (speedup 1.0000x reference)
"""Trainium2 Bass kernel for nn_Align: batched quaternion->rotmat + rigid transform.

reference math (per structure j of 64):
    q = (1, b, c, d) / sqrt(s),  s = 1 + b^2 + c^2 + d^2
    R = rotmat(q)                       # 3x3
    out[j] = pred[j] @ R + t[j]         # [91,3] @ [3,3] + [3]

Sharding: data-parallel over the 8 NeuronCores, 8 structures per core.

Per-core layout: partitions = (structure j:8, point-group g:13) = 104,
free dim = (point-in-group q:7, coord m:3) = 21.

Factorization: R = (2/s)*N - I with N = u (x) u + W, u = (b,c,d),
W = [[1,-d,c],[d,1,-b],[-c,b,1]] (host-packed signed copies), so

    out[q,n] = (2/s) * sum_m X[q,m]*N[m,n]  +  (t[n] - X[q,n]).

DVE pipeline (single engine; scalar stages are all single-element APs and
stream at near-zero marginal cost on the TRN2 DVE):
    N9[3m+n] = u_m*u_n + W[m,n]     9x scalar_tensor_tensor
    S2 = ((b*b/2+.5) + c*c/2) + d*d/2 = s/2   (h = u/2 host-packed)
    IV2 = 1/S2 = 2/s
    TX = t_bcast - X                the -I term plus translation  [21]
    PA[q,n,m] = X[q,m]*N9[m,n]      one 3-free-dim broadcast TT   [63]
    ZN = reduce_m(PA)               innermost-axis reduce   [63 -> 21]
    O  = (ZN * IV2) + TX            scalar_tensor_tensor          [21]

Critical-path engineering (CoreSim cost model):
  - the input lands via a gpsimd SWDGE dma_gather (identity gather, one
    256B row per partition) issued right after the semaphore clears and
    BEFORE the stale-semaphore barrier.  Unlike a DMACopy, the gather's
    sequencer cost is tiny, so the all-engine barrier completes ~400ns
    after launch instead of ~700ns.  The index table is built on-device
    (iota + clamp) and pre-compensated for the gather ucode's 16-entry
    index-stream skip (HW-verified stable: SBUF partition p receives
    index-stream entry p+16).  One production-style load_library(mlp)
    makes DMAGatherAnt available; iota runs first, under the default
    `standard` library.  The gather's completion-sem increment lands far
    after the clears, so clear-before-inc ordering holds with huge margin;
  - all cross-run-stale semaphores are cleared on gpsimd before the
    all-engine barrier; no dma_reset is needed (and with the pre-barrier
    gather it must not run: its drain would wait on the in-flight DMA)
    because every DMA of a run completes before that run's engines drain,
    so no DGE state can leak across NEFF runs;
  - output DMA on the sync engine, gated by one dve_done semaphore hop.
  (A semaphore-free output DMA would additionally hide the DMA-sem
  propagation tail, but walrus requires DGE sync info and a wait-only DGE
  hangs the device — HW-verified unrecoverable; keep full sem sync.)

NOTE: CoreSim's gather executor does not model the ucode's 16-entry skip,
so simulated PK partitions are shifted vs hardware — simulated output
DATA is wrong, but timing and the device path (the one that matters for
correctness) are right.  Correctness is validated against the device.

Raw Bass (no Tile: this walrus build encodes at most one sync-wait per
compute instruction).  Every cross-op RAW dep is semaphore-synced
(streaming same-engine RAW is not safe on HW).
"""

import numpy as np

NCORES = 8
J = 8          # structures per core
G = 13         # point groups per structure
Q = 7          # points per group  (G*Q = 91)
PARTS = J * G  # 104 partitions

# packed row layout (39 floats per (j,g) row):
#   [0:21]  pred, (q,m) interleaved
#   [21:24] u3 = [b c d]
#   [24:33] W9 = [1 -d c  d 1 -b  -c b 1]   (row-major [m,n] addends)
#   [33:36] t
#   [36:39] h3 = u3/2
NPACK = 39
GROWS = 112     # gather rows (104 data + 8 padding, multiple of 16)
GCOLS = 64      # gather row floats (256B elem_size granularity)
C_U3 = 21
C_W9 = 24
C_T = 33
C_H3 = 36

_cache = {}


def _build_nc():
    import concourse.bass as bass
    import concourse.mybir as mybir

    f32 = mybir.dt.float32
    Alu = mybir.AluOpType

    i16 = mybir.dt.int16

    nc = bass.Bass()
    packed = nc.dram_tensor("packed", [GROWS, GCOLS], f32, kind="ExternalInput")
    out = nc.dram_tensor("out", [J, 91, 3], f32, kind="ExternalOutput")

    with (
        nc.sbuf_tensor([128, GCOLS], f32) as PK_t,
        nc.sbuf_tensor([128, 8], i16) as IDX_t,
        nc.sbuf_tensor([PARTS, 9], f32) as N9_t,
        nc.sbuf_tensor([PARTS, 1], f32) as A_t,
        nc.sbuf_tensor([PARTS, 1], f32) as B_t,
        nc.sbuf_tensor([PARTS, 1], f32) as S2_t,
        nc.sbuf_tensor([PARTS, 1], f32) as IV2_t,
        nc.sbuf_tensor([PARTS, 63], f32) as PA_t,
        nc.sbuf_tensor([PARTS, 21], f32) as ZN_t,
        nc.sbuf_tensor([PARTS, 21], f32) as TX_t,
        nc.sbuf_tensor([PARTS, 21], f32) as O_t,
        nc.semaphore("dma_in") as dma_in_sem,
        nc.semaphore("v") as v_sem,
        nc.semaphore("dve_done") as dve_sem,
        nc.semaphore("dma_out") as dma_out_sem,
        nc.semaphore("gx") as gx_sem,
        nc.Block() as block,
    ):
        PK = PK_t[0:PARTS, :]
        O = O_t[:, :]
        X21 = PK[:, 0:21]

        def ucol(m):
            return PK[:, C_U3 + m:C_U3 + m + 1]

        def hcol(m):
            return PK[:, C_H3 + m:C_H3 + m + 1]

        def _pseudo_barrier(eng):
            # NRT expands this to a real all-engine barrier on runtime
            # semaphores outside the kernel sem range — stale-state proof.
            eng.isa(
                nc.isa.Opcode.NEURON_ISA_TPB_OPCODE_PSEUDO_SYNC_BARRIER,
                {},
                struct_name="NEURON_ISA_TPB_UNKNOWN_STRUCT",
                verify=False,
            )

        @block.gpsimd
        def _(gpsimd):
            # Stale-semaphore preamble: semaphores are NOT reset between NEFF
            # executions, and waits here use absolute values.  Clear every sem
            # this kernel waits on or increments, THEN barrier — without the
            # barrier an engine can pass its first wait on a stale value
            # before the clear lands (observed as a HW deadlock).
            nums = sorted(
                x.num
                for x in (dma_in_sem, v_sem, dve_sem, dma_out_sem, gx_sem)
            )
            assert nums[-1] - nums[0] == 4, nums
            gpsimd.sem_clear(range(nums[0], nums[-1] + 1))
            # Identity gather indices, pre-compensated for the gather
            # ucode's 16-entry stream skip (HW-verified stable: SBUF
            # partition p receives index-stream entry p+16; the first 16
            # entries are consumed as pipeline prime/header).  Stream entry
            # s must therefore hold row s-16, clamped to 0 for the 16
            # discarded entries so every table value stays a valid row.
            # iota runs under the default `standard` gpsimd library; then a
            # single production-style load_library(mlp) makes DMAGatherAnt
            # available.
            gpsimd.iota(out=IDX_t[:, :], pattern=[[16, 8]], base=-16,
                        channel_multiplier=1).then_inc(gx_sem, 1)
            gpsimd.wait_ge(gx_sem, 1)
            gpsimd.tensor_scalar(out=IDX_t[:, :], in0=IDX_t[:, :],
                                 scalar1=0, scalar2=GROWS - 1,
                                 op0=Alu.max, op1=Alu.min).then_inc(gx_sem, 1)
            gpsimd.wait_ge(gx_sem, 2)
            from concourse import library_config
            gpsimd.load_library(library_config.mlp)
            gpsimd.dma_gather(
                out_ap=PK_t[:, :].rearrange("p (a e) -> p a e", a=1),
                in_ap=packed[:, :],
                idxs_ap=IDX_t[:, :],
                num_idxs=128,
                num_idxs_reg=128,
                elem_size=GCOLS,
            ).then_inc(dma_in_sem, 16)
            _pseudo_barrier(gpsimd)

        @block.scalar
        def _(scalar):
            _pseudo_barrier(scalar)

        @block.tensor
        def _(tensor):
            _pseudo_barrier(tensor)

        @block.sync
        def _(sync):
            _pseudo_barrier(sync)
            sync.wait_ge(dve_sem, 1)
            sync.dma_start(
                out=out[:, :, :].rearrange("j (g q) m -> (j g) (q m)", g=G),
                in_=O,
            ).then_inc(dma_out_sem, 16)
            sync.wait_ge(dma_out_sem, 16)

        @block.vector
        def _(vector):
            _pseudo_barrier(vector)
            vector.wait_ge(dma_in_sem, 16)

            # Every cross-op RAW dep is sem-synced: each op bumps v_sem,
            # consumers wait on the producer's cumulative count.
            def op(k, *args, **kw):
                return getattr(vector, k)(*args, **kw).then_inc(v_sem, 1)

            # ---- R numerators, one scalar_tensor_tensor per element ----
            # ops 1..9:  N9[3m+n] = u_m * u_n + W[m,n]
            for m in range(3):
                for n in range(3):
                    k = 3 * m + n
                    op("scalar_tensor_tensor", out=N9_t[:, k:k + 1],
                       in0=ucol(m), scalar=ucol(n),
                       in1=PK[:, C_W9 + k:C_W9 + k + 1],
                       op0=Alu.mult, op1=Alu.add)
            # ops 10..13: s/2 then 2/s, all single-element (near-free)
            op("tensor_scalar", out=A_t[:, :], in0=ucol(0), scalar1=hcol(0),  # 10
               scalar2=0.5, op0=Alu.mult, op1=Alu.add)         # bb/2 + 1/2
            vector.wait_ge(v_sem, 10)
            op("scalar_tensor_tensor", out=B_t[:, :], in0=ucol(1),           # 11
               scalar=hcol(1), in1=A_t[:, :], op0=Alu.mult, op1=Alu.add)
            vector.wait_ge(v_sem, 11)
            op("scalar_tensor_tensor", out=S2_t[:, :], in0=ucol(2),          # 12
               scalar=hcol(2), in1=B_t[:, :], op0=Alu.mult, op1=Alu.add)
            vector.wait_ge(v_sem, 12)
            op("reciprocal", out=IV2_t[:, :], in_=S2_t[:, :])  # 2/s         # 13
            # ---- per-point work ----
            op("tensor_tensor",                                              # 14
               out=TX_t[:, :].rearrange("p (q n) -> p q n", n=3),
               in0=PK[:, C_T:C_T + 3].unsqueeze(1).broadcast_to([PARTS, 7, 3]),
               in1=X21.rearrange("p (q n) -> p q n", n=3),
               op=Alu.subtract)                                # TX = t - X
            vector.wait_ge(v_sem, 9)
            # PA[q,n,m] = X[q,m] * N9[m,n]; one op, 3 broadcast free dims.
            op("tensor_tensor",                                              # 15
               out=PA_t[:, :].rearrange("p (q n m) -> p q n m", n=3, m=3),
               in0=PK_t[0:PARTS, 0:21].rearrange("p (q m) -> p q m", m=3)
                   .unsqueeze(2).broadcast_to([PARTS, 7, 3, 3]),
                                                               # X: (q s3,n s0,m s1)
               in1=N9_t[:, 0:9].rearrange("p (m n) -> p n m", n=3)
                   .unsqueeze(1).broadcast_to([PARTS, 7, 3, 3]),
                                                               # N9: (q s0,n s1,m s3)
               op=Alu.mult)
            vector.wait_ge(v_sem, 15)
            op("reduce_sum", out=ZN_t[:, :],                                 # 16
               in_=PA_t[:, :].rearrange("p (q n m) -> p q n m", n=3, m=3),
               axis=mybir.AxisListType.X)                      # sum over m
            vector.wait_ge(v_sem, 16)
            vector.scalar_tensor_tensor(                                     # 17
                out=O, in0=ZN_t[:, :], scalar=IV2_t[:, :], in1=TX_t[:, :],
                op0=Alu.mult, op1=Alu.add,                     # ZN*2/s + (t-X)
            ).then_inc(dve_sem, 1)

    return nc


def get_nc():
    if "nc" not in _cache:
        nc = _build_nc()
        # Raw Bass skips Bacc's codegen pass that fills in .instr bytes for
        # extended InstISA subclasses (the library reload); without it the
        # NEFF compiler fails with "ISA wrong length".
        from concourse.library_overlay import lower_extended_insts

        lower_extended_insts(nc)
        _cache["nc"] = nc
    return _cache["nc"]


def shard_inputs(pred_coor, r_vector, t_vector):
    n = pred_coor.shape[0]
    b, c, d = r_vector[:, 0], r_vector[:, 1], r_vector[:, 2]
    one = np.ones_like(b)
    w9 = np.stack([one, -d, c, d, one, -b, -c, b, one], axis=-1)  # [n,9]
    pk = np.empty((n, G, NPACK), dtype=np.float32)
    pk[:, :, 0:21] = pred_coor.reshape(n, G, 21)
    pk[:, :, C_U3:C_U3 + 3] = r_vector[:, None, :]
    pk[:, :, C_W9:C_W9 + 9] = w9[:, None, :]
    pk[:, :, C_T:C_T + 3] = t_vector[:, None, :]
    pk[:, :, C_H3:C_H3 + 3] = 0.5 * r_vector[:, None, :]
    pk = pk.reshape(n * G, NPACK)
    out_maps = []
    for c in range(NCORES):
        g = np.zeros((GROWS, GCOLS), dtype=np.float32)
        g[:PARTS, :NPACK] = pk[c * PARTS : (c + 1) * PARTS]
        out_maps.append({"packed": g})
    return out_maps


def run(pred_coor, r_vector, t_vector, trace=False):
    from concourse.bass_utils import run_bass_kernel_spmd

    nc = get_nc()
    in_maps = shard_inputs(pred_coor, r_vector, t_vector)
    res = run_bass_kernel_spmd(nc, in_maps, list(range(NCORES)), trace=trace)
    full = np.concatenate([res.results[c]["out"] for c in range(NCORES)], axis=0)
    return full, res


def kernel(pred_coor, r_vector, t_vector):
    pred_coor = np.asarray(pred_coor, dtype=np.float32)
    r_vector = np.asarray(r_vector, dtype=np.float32)
    t_vector = np.asarray(t_vector, dtype=np.float32)
    full, _ = run(pred_coor, r_vector, t_vector, trace=False)
    return full


# revision 17
# speedup vs baseline: 1.0119x; 1.0119x over previous
"""Trainium2 Bass kernel for nn_Align: batched quaternion->rotmat + rigid transform.

reference math (per structure j of 64):
    q = (1, b, c, d) / sqrt(s),  s = 1 + b^2 + c^2 + d^2
    R = rotmat(q)                       # 3x3
    out[j] = pred[j] @ R + t[j]         # [91,3] @ [3,3] + [3]

Sharding: data-parallel over the 8 NeuronCores, 8 structures per core.

Per-core layout: partitions = (structure j:8, point-group g:13) = 104,
free dim = (point-in-group q:7, coord m:3) = 21.

Factorization: R = (2/s)*N - I with N = u (x) u + W, u = (b,c,d),
W = [[1,-d,c],[d,1,-b],[-c,b,1]] (host-packed signed copies), so

    out[q,n] = (2/s) * sum_m4 X4[q,m]*N4[m,n]  -  X[q,n],

where X4 carries a ones column (m=3) and N4's fourth row is (s/2)*t — the
translation rides the same reduction, pre-scaled so the single 2/s
multiply lands everything exactly on X@R + t.

DVE pipeline (single engine; scalar stages are all single-element APs and
stream at near-zero marginal cost on the TRN2 DVE):
    N12[3m+n] = u_m*u_n + W[m,n]    9x scalar_tensor_tensor
    S2 = ((b*b/2+.5) + c*c/2) + d*d/2 = s/2   (h = u/2 host-packed)
    IV2 = 1/S2 = 2/s
    N12[9+n] = S2 * t_n             3x single-element, near-free
    PA[q,n,m4] = X4[q,m]*N12[m,n]   one 3-free-dim broadcast TT   [84]
    ZN = reduce_m4(PA)              innermost-axis reduce   [84 -> 21]
    O  = (ZN * IV2) - X             scalar_tensor_tensor          [21]

Critical-path engineering (CoreSim cost model):
  - the input lands via a gpsimd SWDGE dma_gather (identity gather, one
    256B row per partition) issued right after the semaphore clears and
    BEFORE the stale-semaphore barrier.  Unlike a DMACopy, the gather's
    sequencer cost is tiny, so the all-engine barrier completes ~400ns
    after launch instead of ~700ns.  The index table is built on-device
    (iota + clamp) and pre-compensated for the gather ucode's 16-entry
    index-stream skip (HW-verified stable: SBUF partition p receives
    index-stream entry p+16).  One production-style load_library(mlp)
    makes DMAGatherAnt available; iota runs first, under the default
    `standard` library.  The gather's completion-sem increment lands far
    after the clears, so clear-before-inc ordering holds with huge margin;
  - all cross-run-stale semaphores are cleared on gpsimd before the
    all-engine barrier; no dma_reset is needed (and with the pre-barrier
    gather it must not run: its drain would wait on the in-flight DMA)
    because every DMA of a run completes before that run's engines drain,
    so no DGE state can leak across NEFF runs;
  - output DMA on the sync engine, gated by one dve_done semaphore hop.
  (A semaphore-free output DMA would additionally hide the DMA-sem
  propagation tail, but walrus requires DGE sync info and a wait-only DGE
  hangs the device — HW-verified unrecoverable; keep full sem sync.)

NOTE: CoreSim's gather executor does not model the ucode's 16-entry skip,
so simulated PK partitions are shifted vs hardware — simulated output
DATA is wrong, but timing and the device path (the one that matters for
correctness) are right.  Correctness is validated against the device.

Raw Bass (no Tile: this walrus build encodes at most one sync-wait per
compute instruction).  Every cross-op RAW dep is semaphore-synced
(streaming same-engine RAW is not safe on HW).
"""

import numpy as np

NCORES = 8
J = 8          # structures per core
G = 13         # point groups per structure
Q = 7          # points per group  (G*Q = 91)
PARTS = J * G  # 104 partitions

# packed row layout (46 floats per (j,g) row):
#   [0:28]  pred, (q,m4) interleaved with a ones column at m=3
#   [28:31] u3 = [b c d]
#   [31:40] W9 = [1 -d c  d 1 -b  -c b 1]   (row-major [m,n] addends)
#   [40:43] t
#   [43:46] h3 = u3/2
NPACK = 46
GROWS = 112     # gather rows (104 data + 8 padding, multiple of 16)
GCOLS = 64      # gather row floats (256B elem_size granularity)
C_U3 = 28
C_W9 = 31
C_T = 40
C_H3 = 43

_cache = {}


def _build_nc():
    import concourse.bass as bass
    import concourse.mybir as mybir

    f32 = mybir.dt.float32
    Alu = mybir.AluOpType

    i16 = mybir.dt.int16

    nc = bass.Bass()
    packed = nc.dram_tensor("packed", [GROWS, GCOLS], f32, kind="ExternalInput")
    out = nc.dram_tensor("out", [J, 91, 3], f32, kind="ExternalOutput")

    with (
        nc.sbuf_tensor([128, GCOLS], f32) as PK_t,
        nc.sbuf_tensor([128, 8], i16) as IDX_t,
        nc.sbuf_tensor([PARTS, 12], f32) as N12_t,
        nc.sbuf_tensor([PARTS, 1], f32) as A_t,
        nc.sbuf_tensor([PARTS, 1], f32) as B_t,
        nc.sbuf_tensor([PARTS, 1], f32) as S2_t,
        nc.sbuf_tensor([PARTS, 1], f32) as IV2_t,
        nc.sbuf_tensor([PARTS, 84], f32) as PA_t,
        nc.sbuf_tensor([PARTS, 21], f32) as ZN_t,
        nc.sbuf_tensor([PARTS, 21], f32) as O_t,
        nc.semaphore("dma_in") as dma_in_sem,
        nc.semaphore("v") as v_sem,
        nc.semaphore("dve_done") as dve_sem,
        nc.semaphore("dma_out") as dma_out_sem,
        nc.semaphore("gx") as gx_sem,
        nc.Block() as block,
    ):
        PK = PK_t[0:PARTS, :]
        O = O_t[:, :]
        # X[q, n] view over the (q, m4) packing: q-stride 4, n-stride 1
        X21 = PK[:, 0:28].rearrange("p (q m) -> p q m", m=4)[:, :, 0:3]

        def ucol(m):
            return PK[:, C_U3 + m:C_U3 + m + 1]

        def hcol(m):
            return PK[:, C_H3 + m:C_H3 + m + 1]

        def _pseudo_barrier(eng):
            # NRT expands this to a real all-engine barrier on runtime
            # semaphores outside the kernel sem range — stale-state proof.
            eng.isa(
                nc.isa.Opcode.NEURON_ISA_TPB_OPCODE_PSEUDO_SYNC_BARRIER,
                {},
                struct_name="NEURON_ISA_TPB_UNKNOWN_STRUCT",
                verify=False,
            )

        @block.gpsimd
        def _(gpsimd):
            # Stale-semaphore preamble: semaphores are NOT reset between NEFF
            # executions, and waits here use absolute values.  Clear every sem
            # this kernel waits on or increments, THEN barrier — without the
            # barrier an engine can pass its first wait on a stale value
            # before the clear lands (observed as a HW deadlock).
            nums = sorted(
                x.num
                for x in (dma_in_sem, v_sem, dve_sem, dma_out_sem, gx_sem)
            )
            assert nums[-1] - nums[0] == 4, nums
            gpsimd.sem_clear(range(nums[0], nums[-1] + 1))
            # Identity gather indices, pre-compensated for the gather
            # ucode's 16-entry stream skip (HW-verified stable: SBUF
            # partition p receives index-stream entry p+16; the first 16
            # entries are consumed as pipeline prime/header).  Stream entry
            # s must therefore hold row s-16, clamped to 0 for the 16
            # discarded entries so every table value stays a valid row.
            # iota runs under the default `standard` gpsimd library; then a
            # single production-style load_library(mlp) makes DMAGatherAnt
            # available.
            gpsimd.iota(out=IDX_t[:, :], pattern=[[16, 8]], base=-16,
                        channel_multiplier=1).then_inc(gx_sem, 1)
            gpsimd.wait_ge(gx_sem, 1)
            gpsimd.tensor_scalar(out=IDX_t[:, :], in0=IDX_t[:, :],
                                 scalar1=0, scalar2=GROWS - 1,
                                 op0=Alu.max, op1=Alu.min).then_inc(gx_sem, 1)
            gpsimd.wait_ge(gx_sem, 2)
            from concourse import library_config
            gpsimd.load_library(library_config.mlp)
            gpsimd.dma_gather(
                out_ap=PK_t[:, :].rearrange("p (a e) -> p a e", a=1),
                in_ap=packed[:, :],
                idxs_ap=IDX_t[:, :],
                num_idxs=128,
                num_idxs_reg=128,
                elem_size=GCOLS,
            ).then_inc(dma_in_sem, 16)
            _pseudo_barrier(gpsimd)

        @block.scalar
        def _(scalar):
            _pseudo_barrier(scalar)

        @block.tensor
        def _(tensor):
            _pseudo_barrier(tensor)

        @block.sync
        def _(sync):
            _pseudo_barrier(sync)
            sync.wait_ge(dve_sem, 1)
            sync.dma_start(
                out=out[:, :, :].rearrange("j (g q) m -> (j g) (q m)", g=G),
                in_=O,
            ).then_inc(dma_out_sem, 16)
            sync.wait_ge(dma_out_sem, 16)

        @block.vector
        def _(vector):
            _pseudo_barrier(vector)
            vector.wait_ge(dma_in_sem, 16)

            # Every cross-op RAW dep is sem-synced: each op bumps v_sem,
            # consumers wait on the producer's cumulative count.
            def op(k, *args, **kw):
                return getattr(vector, k)(*args, **kw).then_inc(v_sem, 1)

            # ---- R numerators, one scalar_tensor_tensor per element ----
            # ops 1..9:  N12[3m+n] = u_m * u_n + W[m,n]   (rows m = 0..2)
            for m in range(3):
                for n in range(3):
                    k = 3 * m + n
                    op("scalar_tensor_tensor", out=N12_t[:, k:k + 1],
                       in0=ucol(m), scalar=ucol(n),
                       in1=PK[:, C_W9 + k:C_W9 + k + 1],
                       op0=Alu.mult, op1=Alu.add)
            # ops 10..13: s/2 then 2/s, all single-element (near-free)
            op("tensor_scalar", out=A_t[:, :], in0=ucol(0), scalar1=hcol(0),  # 10
               scalar2=0.5, op0=Alu.mult, op1=Alu.add)         # bb/2 + 1/2
            vector.wait_ge(v_sem, 10)
            op("scalar_tensor_tensor", out=B_t[:, :], in0=ucol(1),           # 11
               scalar=hcol(1), in1=A_t[:, :], op0=Alu.mult, op1=Alu.add)
            vector.wait_ge(v_sem, 11)
            op("scalar_tensor_tensor", out=S2_t[:, :], in0=ucol(2),          # 12
               scalar=hcol(2), in1=B_t[:, :], op0=Alu.mult, op1=Alu.add)
            vector.wait_ge(v_sem, 12)
            op("reciprocal", out=IV2_t[:, :], in_=S2_t[:, :])  # 2/s         # 13
            # ops 14..16: N12 row m=3 = (s/2) * t_n, so the reduce折 carries
            # the translation pre-scaled and O = ZN*(2/s) - X lands exactly
            # on X@R + t.  Single-element, near-free.
            for n in range(3):
                op("tensor_scalar", out=N12_t[:, 9 + n:10 + n],
                   in0=PK[:, C_T + n:C_T + n + 1], scalar1=S2_t[:, :],
                   scalar2=None, op0=Alu.mult)
            vector.wait_ge(v_sem, 16)
            # PA[q,n,m4] = X4[q,m] * N12[m,n]; one op, 3 broadcast free dims.
            op("tensor_tensor",                                              # 17
               out=PA_t[:, :].rearrange("p (q n m) -> p q n m", n=3, m=4),
               in0=PK_t[0:PARTS, 0:28].rearrange("p (q m) -> p q m", m=4)
                   .unsqueeze(2).broadcast_to([PARTS, 7, 3, 4]),
                                                               # X4: (q s4,n s0,m s1)
               in1=N12_t[:, 0:12].rearrange("p (m n) -> p n m", n=3)
                   .unsqueeze(1).broadcast_to([PARTS, 7, 3, 4]),
                                                               # N12: (q s0,n s1,m s3)
               op=Alu.mult)
            vector.wait_ge(v_sem, 17)
            op("reduce_sum", out=ZN_t[:, :],                                 # 18
               in_=PA_t[:, :].rearrange("p (q n m) -> p q n m", n=3, m=4),
               axis=mybir.AxisListType.X)                      # sum over m4
            vector.wait_ge(v_sem, 18)
            vector.scalar_tensor_tensor(                                     # 19
                out=O, in0=ZN_t[:, :], scalar=IV2_t[:, :], in1=X21,
                op0=Alu.mult, op1=Alu.subtract,                # ZN*2/s - X
            ).then_inc(dve_sem, 1)

    return nc


def get_nc():
    if "nc" not in _cache:
        nc = _build_nc()
        # Raw Bass skips Bacc's codegen pass that fills in .instr bytes for
        # extended InstISA subclasses (the library reload); without it the
        # NEFF compiler fails with "ISA wrong length".
        from concourse.library_overlay import lower_extended_insts

        lower_extended_insts(nc)
        _cache["nc"] = nc
    return _cache["nc"]


def shard_inputs(pred_coor, r_vector, t_vector):
    n = pred_coor.shape[0]
    b, c, d = r_vector[:, 0], r_vector[:, 1], r_vector[:, 2]
    one = np.ones_like(b)
    w9 = np.stack([one, -d, c, d, one, -b, -c, b, one], axis=-1)  # [n,9]
    pk = np.empty((n, G, NPACK), dtype=np.float32)
    pq = pk[:, :, 0:28].reshape(n, G, 7, 4)
    pq[:, :, :, 0:3] = pred_coor.reshape(n, G, 7, 3)
    pq[:, :, :, 3] = 1.0
    pk[:, :, C_U3:C_U3 + 3] = r_vector[:, None, :]
    pk[:, :, C_W9:C_W9 + 9] = w9[:, None, :]
    pk[:, :, C_T:C_T + 3] = t_vector[:, None, :]
    pk[:, :, C_H3:C_H3 + 3] = 0.5 * r_vector[:, None, :]
    pk = pk.reshape(n * G, NPACK)
    out_maps = []
    for c in range(NCORES):
        g = np.zeros((GROWS, GCOLS), dtype=np.float32)
        g[:PARTS, :NPACK] = pk[c * PARTS : (c + 1) * PARTS]
        out_maps.append({"packed": g})
    return out_maps


def run(pred_coor, r_vector, t_vector, trace=False):
    from concourse.bass_utils import run_bass_kernel_spmd

    nc = get_nc()
    in_maps = shard_inputs(pred_coor, r_vector, t_vector)
    res = run_bass_kernel_spmd(nc, in_maps, list(range(NCORES)), trace=trace)
    full = np.concatenate([res.results[c]["out"] for c in range(NCORES)], axis=0)
    return full, res


def kernel(pred_coor, r_vector, t_vector):
    pred_coor = np.asarray(pred_coor, dtype=np.float32)
    r_vector = np.asarray(r_vector, dtype=np.float32)
    t_vector = np.asarray(t_vector, dtype=np.float32)
    full, _ = run(pred_coor, r_vector, t_vector, trace=False)
    return full
